# revision 55
# baseline (speedup 1.0000x reference)
"""Causal self-attention (B=1, T=4096, C=1024, H=8) on 8 trn2 NeuronCores.

Tensor-parallel over heads: core h owns head h (D=128 = partition width).
Everything is computed feature-major ("transposed") so the PE contraction
dim always sits on SBUF partitions:

  per core h:
    qT,kT = [d, t] = Wq/Wk_h @ x.T      (PE, contraction over c)
    v     = [t, d]                       (swapped-operand matmul)
    attT  = [s, t] = kT.T-blocks @ qT    (scores, transposed)
    p     = exp(attT)                    (ACT; no max-subtraction --
                                          logits are O(3) for this data)
    mask: DVE multiply by precomputed 0/1 tiles on diagonal-crossing tiles
    P_acc += p                           (DVE elementwise accumulate over
                                          s-tiles; replaces the per-s-tile
                                          ones-matmul denominator --
                                          saves ~45us of PE time)
    sums  = ones[128,128].T @ P_acc      (PE; one matmul per chunk-half
                                          replicates the denominator to
                                          all partitions)
    yTu   = v.T-blocks @ p               (PE accumulate over s-tiles)
    yT    = yTu * (1/sums)               (DVE fast-reciprocal + mul)
    outP  = Wp[:, head-cols].T-blocks @ yT   (LOCAL partial of the full
                                          c_proj -- no collective; the
                                          output is sum-sharded, bf16)
  host: sum the 8 partials, add b_eff, transpose -> [1, T, C]

  Bias structure exploited:
   - k bias: dropped entirely (softmax over s is invariant to the
     per-query constant q~ . bk).
   - v bias: folded into the output bias on host (softmax rows sum to 1,
     so bv passes straight through attention: b_eff = b_proj + Wp @ bv).

  Scheduling (the big wins over the naive emission, 250us -> ~162us):
   - software-pipelined attention loop: scores(si+1) + exp(si+1) emitted
     before AV(si), so the ACT exp latency hides behind PE matmuls
     instead of stalling them per s-tile.
   - c_proj of pair pc is interleaved one j-tile per s-tile body of pair
     pc+1's attention: its matmuls fill PE's exp-wait slots and its
     PSUM-drain copies never lump up in front of the next pair's
     exp/accumulate queues.
   - next pair's QKV is emitted before this pair's c_proj (scores depend
     on QKV; c_proj only feeds the output DMA).
   - QKV accumulation groups run halves-outer so 8 consecutive matmuls
     hit the same PSUM bank (bank-alternating accumulation measured
     ~165ns/MM slower from PE micro-idles).
   - diagonal-crossing s-tiles slice off the sub-diagonal columns
     (valid region [c0, T2) is contiguous), so the causal wedge costs no
     PE/ACT/DVE work and the mask shrinks to one shared 128x128 block.
   - 64 warmup matmuls bridge the input-DMA latency so the PE HAM
     clock-gate opens (K=8/8 = 2.4GHz) before the first real matmul.
   - every DMA transfer is one per-partition-contiguous block (strided
     transfers degrade to small descriptors), and output tiles are
     batched 4-to-a-transfer (per-dma_start fixed cost ~2us).

  (An AllGather + column-shard variant was measured slower: the ~85us
  cross-core launch skew of the 8-device dispatch lands on whichever
  core waits for the last collective piece.)
"""

import math
import os
import sys

for _p in ("/opt/trn_rl_repo",):
    if _p not in sys.path:
        sys.path.insert(0, _p)

import numpy as np
import ml_dtypes

import concourse.bass as bass
import concourse.mybir as mybir
import concourse.tile as tile
from concourse import bacc
from concourse import bass_utils
from concourse.masks import make_identity

B, T, C, H = 1, 4096, 1024, 8
D = C // H          # 128, head dim == partition width
N_CORES = 8
TQ = 512            # query-chunk (matmul moving free dim)
CO = C // 128       # 8 contraction tiles of 128
F32 = mybir.dt.float32
BF16 = mybir.dt.bfloat16

# dtype knobs
MM_DT = BF16        # qkv/proj matmul operand + v / weight storage
P_DT = BF16         # qT/kT storage and exp(att) storage
AG_DT = BF16        # yT storage
XT_DT = BF16        # x.T input payload
OUT_DT = BF16       # outP partial payload (summed in f32 on host)


def _np_dt(dt):
    return {F32: np.float32, BF16: ml_dtypes.bfloat16}[dt]


def build(t_len=T, mm_dt=MM_DT, p_dt=P_DT, ag_dt=AG_DT, xt_dt=XT_DT):
    """Emit the single-core SPMD program (same code on all 8 cores)."""
    n_chunks = t_len // TQ
    n_pairs = n_chunks // 2   # query chunks processed in pairs of 2*TQ cols
    n_ttiles = t_len // 128
    nc = bacc.Bacc(
        "TRN2", target_bir_lowering=False, debug=False, num_devices=N_CORES
    )

    # All DRAM tensors are laid out so every DMA transfer is one fully
    # contiguous block (strided transfers degrade to 1-2KB descriptors and
    # run ~5x slower); the host does the re-tiling.
    n_pairs_ = t_len // (2 * TQ)
    n_chunks_ = t_len // TQ
    xT_d = nc.dram_tensor("xT", [n_pairs_, 2, 128, (CO // 2) * 2 * TQ], xt_dt,
                          kind="ExternalInput")
    wq_d = nc.dram_tensor("wq", [128, CO * D], mm_dt, kind="ExternalInput")
    wk_d = nc.dram_tensor("wk", [128, CO * D], mm_dt, kind="ExternalInput")
    wv_d = nc.dram_tensor("wv", [128, CO * D], mm_dt, kind="ExternalInput")
    wp_d = nc.dram_tensor("wp", [D, CO, 128], mm_dt, kind="ExternalInput")
    bq_d = nc.dram_tensor("bq", [D, 1], F32, kind="ExternalInput")
    # output groups 4 j-tiles per DMA: 512KB contiguous transfers with
    # 4KB-per-partition descriptors (per-tile DMAs are fixed-cost bound)
    OG = 4
    outP_d = nc.dram_tensor("outP", [n_chunks_, CO // OG, 128, OG * TQ],
                            OUT_DT, kind="ExternalOutput")

    with tile.TileContext(nc) as tc:
        with (
            tc.tile_pool(name="const", bufs=1) as cpool,
            tc.tile_pool(name="persist", bufs=1) as ppool,
            tc.tile_pool(name="work", bufs=2) as wpool,
            tc.tile_pool(name="ptiles", bufs=5) as pt_pool,
            tc.tile_pool(name="psum", bufs=1, space="PSUM") as psum,
            tc.tile_pool(name="dram", bufs=1, space="DRAM") as dram,
        ):
            # ---- constants / weights -------------------------------------
            # ones first: the HAM/ifetch warmup matmuls depend only on it,
            # so PE starts as early as possible
            ones_sq = cpool.tile([128, 128], p_dt, name="ones_sq")
            nc.vector.memset(ones_sq[:], 1.0)
            # HAM warmup: dummy matmuls bridge until the first inputs land
            # (~12us incl preamble), so the PE clock-gate opens (K=8/8)
            # before the first real matmul and never re-throttles
            warm_ps = psum.tile([128, 128], F32, tag="s2", name="warm_ps", bufs=2)
            for wi in range(64):
                nc.tensor.matmul(warm_ps[:], ones_sq[:], ones_sq[:],
                                 start=True, stop=True)
            # per-o wq/x transfers interleaved so the o=0 matmul is
            # unblocked after two transfers instead of all of them
            wq_sb = cpool.tile([128, CO, D], mm_dt, name="wq_sb")
            wk_sb = cpool.tile([128, CO, D], mm_dt, name="wk_sb")
            wv_sb = cpool.tile([128, CO, D], mm_dt, name="wv_sb")
            wp_sb = cpool.tile([128, CO, D], mm_dt, name="wp_sb")
            bq_sb = cpool.tile([D, 1], F32, name="bq_sb")
            # single 128x128 triangular mask (keep t >= s): with wedge
            # slicing every diagonal-crossing tile masks only its own
            # 128-wide diagonal block, and that block is the same for all
            dmask = cpool.tile([128, 128], p_dt, name="dmask")
            nc.vector.memset(dmask[:], 1.0)
            nc.gpsimd.affine_select(
                out=dmask[:], in_=dmask[:],
                compare_op=mybir.AluOpType.is_ge, fill=0.0,
                base=0, pattern=[[1, 128]], channel_multiplier=-1,
            )
            ident = cpool.tile([128, 128], p_dt, name="ident")
            make_identity(nc, ident[:])

            # ---- persistent activations ----------------------------------
            kT_sb = ppool.tile([128, t_len], p_dt, name="kT_sb")
            v_sb = ppool.tile([128, n_ttiles, D], mm_dt, name="v_sb")
            yT_sb = ppool.tile([128, t_len], ag_dt, name="yT_sb")

            T2 = 2 * TQ

            def dma_x(xc, pc):
                # two 1MB transfers per pair: per-partition-contiguous
                # blocks use all SDMA engines at full rate
                h = CO // 2
                for i in range(2):
                    nc.sync.dma_start(
                        xc[:, i * h : (i + 1) * h, :].rearrange(
                            "p o t -> p (o t)"),
                        xT_d.ap()[pc, i],
                    )

            xc0 = wpool.tile([128, CO, T2], xt_dt, tag="xc", name="xc0", bufs=2)
            nc.sync.dma_start(
                wq_sb[:].rearrange("p o m -> p (o m)"), wq_d.ap()
            )
            dma_x(xc0, 0)
            nc.sync.dma_start(bq_sb[:], bq_d.ap())
            nc.sync.dma_start(
                wk_sb[:].rearrange("p o m -> p (o m)"), wk_d.ap()
            )
            nc.sync.dma_start(
                wv_sb[:].rearrange("p o m -> p (o m)"), wv_d.ap()
            )
            nc.sync.dma_start(
                wp_sb[:], wp_d.ap().rearrange("d o j -> d (o j)")
            )

            def build_cproj_units(pj):
                # c_proj for pair pj as 16 closures, interleaved one per
                # s-tile body of the NEXT pair's attention: fills PE's
                # exp-wait slots and keeps the drain copies from lumping
                # up in front of the next pair's exp/accumulate queues.
                units = []
                box = {}

                def unit(half, g, jj, ui, tail=False):
                    ck = pj * 2 + half
                    lo = ck * TQ
                    j = g * OG + jj
                    if jj == 0:
                        box[(half, g)] = wpool.tile(
                            [128, OG * TQ], OUT_DT, tag="outc",
                            name="outc", bufs=3)
                    outc = box[(half, g)]
                    oh = psum.tile([128, TQ], F32, tag="aux", name="oh",
                                   bufs=2)
                    nc.tensor.matmul(
                        oh[:], wp_sb[:, j, :], yT_sb[:, lo : lo + TQ],
                        start=True, stop=True,
                    )
                    oc = outc[:, jj * TQ : (jj + 1) * TQ]
                    # interleaved: ALL on DVE -- ACT's exp is within 8%
                    # of pacing the attention loop, so any drain in its
                    # FIFO delays the exp chain; tail (no exp pressure):
                    # alternate evenly
                    on_act = (ui % 2 == 0) if tail else False
                    if on_act:
                        nc.scalar.copy(oc, oh[:])
                    else:
                        nc.vector.tensor_copy(oc, oh[:])
                    if tail:
                        # smaller tail transfers: the last DMA is serial
                        # with kernel end, so don't batch it
                        if jj % 2 == 1:
                            sl_ = slice((jj - 1) * TQ, (jj + 1) * TQ)
                            nc.sync.dma_start(
                                outP_d.ap()[ck, g][:, sl_], outc[:, sl_])
                    elif jj == OG - 1:
                        nc.sync.dma_start(outP_d.ap()[ck, g], outc[:])

                ui = 0
                for half in range(2):
                    for g in range(CO // OG):
                        for jj in range(OG):
                            units.append(
                                lambda h=half, g_=g, m=jj, u=ui, t=False: unit(h, g_, m, u, t)
                            )
                            ui += 1
                return units

            def emit_qkv(pc, after_q2=None):
                """QKV projections for pair pc; returns (qT_cur, vT_tmp)."""
                t0 = pc * T2
                if pc == 0:
                    xc = xc0
                else:
                    xc = wpool.tile([128, CO, T2], xt_dt, tag="xc", name="xc", bufs=2)
                    dma_x(xc, pc)

                # halves OUTER: 8 consecutive matmuls hit the same PSUM
                # bank (bank-alternating accumulation measured ~165ns/MM
                # slower from PE micro-idles)
                q2 = psum.tile([128, T2], F32, tag="s2", name="q2", bufs=2)
                for half in range(2):
                    hs = slice(half * TQ, (half + 1) * TQ)
                    for o in range(CO):
                        nc.tensor.matmul(
                            q2[:, hs], wq_sb[:, o, :], xc[:, o, hs],
                            start=(o == 0), stop=(o == CO - 1),
                        )
                if after_q2 is not None:
                    after_q2()
                qT_cur = wpool.tile([128, T2], p_dt, tag="qT", name="qT_cur", bufs=2)
                nc.vector.tensor_add(
                    qT_cur[:], q2[:], bq_sb[:, 0:1].to_broadcast([D, T2])
                )
                k2 = psum.tile([128, T2], F32, tag="s2", name="k2", bufs=2)
                for half in range(2):
                    hs = slice(half * TQ, (half + 1) * TQ)
                    for o in range(CO):
                        nc.tensor.matmul(
                            k2[:, hs], wk_sb[:, o, :], xc[:, o, hs],
                            start=(o == 0), stop=(o == CO - 1),
                        )
                # v: feature-major matmul (wide, shared weights) then PE
                # transpose to token-major
                v2 = psum.tile([128, T2], F32, tag="s2", name="v2", bufs=2)
                for half in range(2):
                    hs = slice(half * TQ, (half + 1) * TQ)
                    for o in range(CO):
                        nc.tensor.matmul(
                            v2[:, hs], wv_sb[:, o, :], xc[:, o, hs],
                            start=(o == 0), stop=(o == CO - 1),
                        )
                vT_tmp = wpool.tile([128, T2], p_dt, tag="vT", name="vT_tmp", bufs=2)
                nc.vector.tensor_copy(vT_tmp[:], v2[:])   # bv folded on host
                # kT copyback (no bias: softmax is invariant to q.bk)
                nc.vector.tensor_copy(kT_sb[:, t0 : t0 + T2], k2[:])
                return qT_cur, vT_tmp

            qkv_cur = emit_qkv(0)
            pending_cproj = []

            for pc in range(n_pairs):
                t0 = pc * T2           # start of chunk A; chunk B at t0+TQ
                qT_cur, vT_tmp = qkv_cur

                def emit_transposes():
                    vt_ps = psum.tile([128, 8, 128], p_dt, tag="aux",
                                      name="vt_ps", bufs=2)
                    for tt in range(8):
                        col = tt * 128
                        nc.tensor.transpose(
                            vt_ps[:, tt, :], vT_tmp[:, col : col + 128], ident[:]
                        )
                    nc.vector.tensor_copy(
                        v_sb[:, pc * 8 : pc * 8 + 8, :], vt_ps[:]
                    )

                # ---- attention for the pair (software-pipelined) ---------
                n_sA = (t0 + TQ) // 128        # s-tiles for chunk A
                n_sB = (t0 + T2) // 128        # s-tiles for chunk B
                yAB = psum.tile([128, T2], F32, tag="yAB", name="yAB", bufs=1)
                A, Bh = slice(0, TQ), slice(TQ, T2)
                recip = wpool.tile([128, T2], F32, tag="recip", name="recip", bufs=2)
                P_acc = wpool.tile([128, T2], p_dt, tag="pacc", name="P_acc", bufs=2)

                def hinfo(si):
                    # active halves as (base, c0, masked): valid cols are
                    # [base+c0, base+TQ); masked = 128-wide diagonal block
                    # at base+c0 needs the triangular mask. Skipping cols
                    # below the diagonal (c0>0) trims the causal wedge from
                    # PE/ACT/DVE work; A- and B-half boundary tiles are
                    # mutually exclusive since n_sB = n_sA + 4.
                    out = []
                    if si < n_sA:
                        j = si - (n_sA - 4)
                        out.append((0, 128 * j if j >= 0 else 0, j >= 0))
                    j = si - (n_sB - 4)
                    out.append((TQ, 128 * j if j >= 0 else 0, j >= 0))
                    return out

                def emit_scores(si):
                    s0 = si * 128
                    s2 = psum.tile([128, T2], F32, tag="s2", name="s2", bufs=2)
                    for base, c0, _m in hinfo(si):
                        nc.tensor.matmul(
                            s2[:, base + c0 : base + TQ],
                            kT_sb[:, s0 : s0 + 128],
                            qT_cur[:, base + c0 : base + TQ],
                            start=True, stop=True)
                    return s2

                def emit_exp(si, s2):
                    # valid region [sc, T2) is contiguous: one exp
                    p2 = pt_pool.tile([128, T2], p_dt, tag="p2", name="p2")
                    hs = hinfo(si)
                    sc = hs[0][0] + hs[0][1]
                    nc.scalar.activation(
                        p2[:, sc:T2], s2[:, sc:T2],
                        mybir.ActivationFunctionType.Exp)
                    return p2

                def emit_masks(si, p2):
                    for base, c0, m in hinfo(si):
                        if m:
                            nc.vector.tensor_mul(
                                p2[:, base + c0 : base + c0 + 128],
                                p2[:, base + c0 : base + c0 + 128],
                                dmask[:])

                # prologue: stage si=0
                s2c = emit_scores(0)
                p2c = emit_exp(0, s2c)
                emit_masks(0, p2c)
                if pc == 0:
                    emit_transposes()   # pair 0's AV needs own v from si=0

                for si in range(n_sB):
                    in_A = si < n_sA
                    p2 = p2c
                    if si + 1 < n_sB:
                        s2c = emit_scores(si + 1)
                        p2c = emit_exp(si + 1, s2c)
                    # denominator accumulate on DVE (replaces PE ones-matmul)
                    hs_i = hinfo(si)
                    sc_i = hs_i[0][0] + hs_i[0][1]
                    if si == 0:
                        nc.vector.tensor_copy(P_acc[:], p2[:])
                    else:
                        nc.vector.tensor_add(P_acc[:, sc_i:T2],
                                             P_acc[:, sc_i:T2],
                                             p2[:, sc_i:T2])
                    if si + 1 < n_sB:
                        emit_masks(si + 1, p2c)
                    if pc > 0 and si == 3:
                        # own-pair v only needed from si >= 8*pc; transposing
                        # here hides the vT copyback latency behind scores
                        emit_transposes()
                    for base, c0, _m in hs_i:
                        n_sX = n_sA if base == 0 else n_sB
                        nc.tensor.matmul(
                            yAB[:, base + c0 : base + TQ], v_sb[:, si, :],
                            p2[:, base + c0 : base + TQ],
                            start=(si == 0), stop=(si == n_sX - 1),
                            skip_group_check=True)
                    if pending_cproj:
                        pending_cproj.pop(0)()
                    if si == min(n_sA + 1, n_sB - 1):
                        # A-half reduce + normalize early so c_proj's A-half
                        # matmuls are unblocked the moment the pair ends
                        # (delayed 2 s-tiles past n_sA-1 so PE doesn't stall
                        # waiting for the DVE accumulate to catch up)
                        sums_a = psum.tile([128, TQ], F32, tag="aux",
                                           name="sums_a", bufs=2)
                        nc.tensor.matmul(sums_a[:], ones_sq[:], P_acc[:, A],
                                         start=True, stop=True)
                        nc.vector.reciprocal_approx_fast(recip[:, A], sums_a[:])
                        nc.vector.tensor_mul(
                            yT_sb[:, t0 : t0 + TQ], yAB[:, A], recip[:, A]
                        )

                def emit_sums_b():
                    # B-half reduce: emitted after the next pair's q2
                    # matmuls so the DVE accumulate of the last s-tiles
                    # has caught up by the time PE reaches it
                    sums_b = psum.tile([128, TQ], F32, tag="aux",
                                       name="sums_b", bufs=2)
                    nc.tensor.matmul(sums_b[:], ones_sq[:], P_acc[:, Bh],
                                     start=True, stop=True)
                    nc.vector.reciprocal_approx_fast(recip[:, Bh], sums_b[:])
                    nc.vector.tensor_mul(
                        yT_sb[:, t0 + TQ : t0 + T2], yAB[:, Bh], recip[:, Bh]
                    )

                # next pair's QKV before this pair's c_proj: those matmuls
                # are on the critical path (scores depend on them) while
                # c_proj only feeds the output DMA; this also queues the
                # next qT/kT drains ahead of the c_proj drain copies on DVE
                # flush any c_proj units the body loop didn't absorb
                for u in pending_cproj:
                    u()
                pending_cproj = build_cproj_units(pc)

                if pc + 1 < n_pairs:
                    qkv_cur = emit_qkv(pc + 1, after_q2=emit_sums_b)
                else:
                    # A-half c_proj is ready now; emit before sums_b so
                    # PE has work while DVE finishes the last accumulates
                    for u in pending_cproj[:8]:
                        u(t=True)
                    emit_sums_b()
                    for u in pending_cproj[8:]:
                        u(t=True)
                    pending_cproj = []


    nc.compile()
    return nc


def make_in_maps(x, w_attn, b_attn, w_proj, b_proj, t_len=T,
                 mm_dt=MM_DT, xt_dt=XT_DT):
    """Shard + lay out the full inputs for the 8 cores."""
    x = np.asarray(x, dtype=np.float32).reshape(t_len, C)
    w_attn = np.asarray(w_attn, dtype=np.float32)
    b_attn = np.asarray(b_attn, dtype=np.float32)
    w_proj = np.asarray(w_proj, dtype=np.float32)

    scale = 1.0 / math.sqrt(D)
    mm_np = _np_dt(mm_dt)
    # x.T pre-tiled into per-partition-contiguous [pair, half, 128, ...]
    # DMA blocks (half = 4 c-tiles)
    n_pairs = t_len // (2 * TQ)
    xT = np.ascontiguousarray(
        x.T.reshape(2, CO // 2, 128, n_pairs, 2 * TQ)
        .transpose(3, 0, 2, 1, 4)
        .reshape(n_pairs, 2, 128, (CO // 2) * 2 * TQ)
    ).astype(_np_dt(xt_dt))

    def wtile(w):  # [C, D] -> per-partition-contiguous [128, CO*D]
        return np.ascontiguousarray(
            w.reshape(CO, 128, D).transpose(1, 0, 2).reshape(128, CO * D)
        ).astype(mm_np)

    in_maps = []
    for h in range(N_CORES):
        sl = slice(h * D, (h + 1) * D)
        wq = wtile((w_attn[sl, :] * scale).T)
        wk = wtile(w_attn[C + h * D : C + (h + 1) * D, :].T)
        wv = wtile(w_attn[2 * C + h * D : 2 * C + (h + 1) * D, :].T)
        wp = np.ascontiguousarray(w_proj[:, sl].T.reshape(D, CO, 128)).astype(mm_np)
        in_maps.append({
            "xT": xT,
            "wq": wq, "wk": wk, "wv": wv, "wp": wp,
            "bq": (b_attn[sl] * scale).reshape(D, 1).astype(np.float32),
        })
    return in_maps


_COMPILED = {}


def _get_compiled(t_len=T):
    if t_len not in _COMPILED:
        _COMPILED[t_len] = build(t_len)
    return _COMPILED[t_len]


def kernel(x, w_attn, b_attn, w_proj, b_proj, trace=False):
    nc = _get_compiled()
    in_maps = make_in_maps(x, w_attn, b_attn, w_proj, b_proj)
    res = bass_utils.run_bass_kernel_spmd(
        nc, in_maps, core_ids=list(range(N_CORES)), trace=trace
    )
    acc = res.results[0]["outP"].astype(np.float32)
    for h in range(1, N_CORES):
        acc += res.results[h]["outP"].astype(np.float32)
    # outP layout: [chunk, j-group, p, jj*TQ+t]  ->  [c, t]
    n_chunks = T // TQ
    OG = 4
    acc = acc.reshape(n_chunks, CO // OG, 128, OG, TQ)
    acc = acc.transpose(1, 3, 2, 0, 4).reshape(C, T)
    # bv passes through attention (softmax rows sum to 1): fold into bias
    b_attn = np.asarray(b_attn, dtype=np.float32)
    bv = b_attn[2 * C : 3 * C]
    b_eff = np.asarray(b_proj, dtype=np.float32) + \
        np.asarray(w_proj, dtype=np.float32) @ bv
    out = acc.T + b_eff
    out = np.ascontiguousarray(out, dtype=np.float32).reshape(B, T, C)
    if trace:
        kernel.last_exec_time_ns = res.exec_time_ns
        kernel.last_results = res
    return out
